# revision 53
# baseline (speedup 1.0000x reference)
"""Trainium2 Bass kernel for CausalMessagePassingLayer (2x GCN + gated scatter).

Sharding: 8 cores = 4 samples x 2 halves of the embedding dim (D=768 -> 384).
Each core is fully independent (no collectives).

Pipeline (graph renumbering kills all gathers / DRAM bounces):
  - host renumbers each subgraph's nodes so that tokens present in BOTH
    images sit at the SAME row position in the edges-GCN and nodes-GCN
    output spaces (class-i rows at positions 0..|i|-1 of both).  The final
    combine then reads both GCN outputs as contiguous SBUF tiles - no
    dma_gather, no DRAM roundtrip for e_nodes, no serial valley.
  - xw: y = dinv_src * (x @ W[:, half]); fp8 DoubleRow for the nodes GCN
    (error budget allows one GCN in fp8; max-err elements are e-driven),
    bf16 for the edges GCN; xT is piece-loaded (contiguous-per-partition
    DRAM layout) so the first matmul starts ~12us in; PSUM drains
    alternate scalar/vector; y kept in SBUF as fp8
  - GCN aggregation z[dst] += count * y[src] as dense per-tile adjacency
    blocks (fp8 counts, incl. self-loops) with DoubleRow pairs of src
    tiles accumulating in PSUM; epilogue -> e rows bf16 in SBUF.  This is
    the PE floor; gather-based sparse alternatives lose on DMA descriptor
    rate (~70ns/row/queue).
  - output rows: region A = all 4096 edges-image rows (out = t + e_e +
    mask*e_n, mask zeroes e_n for rows whose token is not in the nodes
    image); region B = nodes-row chunks >= LI0 (out = t + e_n), covering
    all nodes-only tokens (duplicated class-i rows are dropped on host)
  - region-B waves and region-A pre-sums (p = t + mask*e_n) are emitted
    inside the nodes agg loop as each chunk group completes, so the
    vector queue and in-order sync sequencer are drained by agg_n's end;
    each region-A wave inside the edges agg loop is then a single add
    p + e_e; och out-writes are issued from the (otherwise idle) gpsimd
    queue so they never block the sync sequencer's At stream; the next
    GCN's xT piece loads are interleaved into the agg loop for the same
    reason
  - out rows are written bf16 (host converts back to f32); tokens in
    neither image are exact f32 t passthrough assembled on host

Host-side work is restricted to index/descriptor preparation (renumbering,
dense count blocks, degree counts, masks, inverse permutations) and
dtype/layout marshalling of inputs; all floating-point math runs on device.
"""

import numpy as np
import ml_dtypes

B, S, D, N, E = 4, 8192, 768, 4096, 32768
H = D // 2            # per-core half of embedding dim
P = 128
NT = N // P           # 32 node tiles per graph
KT = D // P           # 6 k-tiles of the contraction dim
FINW = 4              # chunks per final wave

NXC = 16              # xT load pieces (256 columns each)
XW_FP8 = {"n": True, "e": False}   # fp8 DoubleRow xw per GCN (error budget)

bf16 = ml_dtypes.bfloat16
f8 = ml_dtypes.float8_e4m3

# test-harness knobs (the grading harness just calls kernel() and these stay default)
TRACE = False
TRACE_CORES = None
LAST_RESULT = None


def _prep_graph(ei):
    """Dense adjacency-count blocks (incl. self loops) + degree counts.

    Returns (A_blocks, deg): A_blocks[t, p, sc, q] = #edges src=sc*128+p ->
    dst=t*128+q, laid out so A_blocks[t] is directly the stack of matmul
    lhsT tiles for dst-tile t. deg includes the self loop.
    """
    s = np.concatenate([ei[0].astype(np.int64), np.arange(N, dtype=np.int64)])
    d = np.concatenate([ei[1].astype(np.int64), np.arange(N, dtype=np.int64)])
    A = np.zeros((N, N), np.float32)
    np.add.at(A, (d, s), 1.0)
    deg = np.bincount(d, minlength=N).astype(np.int32)
    Ab = np.ascontiguousarray(
        A.reshape(NT, P, NT, P).transpose(0, 3, 2, 1)
    ).astype(f8)
    return Ab, deg


def _xT(x):
    """[N, D] -> bf16 [P, KT, N] with K element d = s*128 + p."""
    return np.ascontiguousarray(x.T.reshape(KT, P, N).transpose(1, 0, 2)).astype(bf16)


def kernel(**inputs):
    import concourse.bacc as bacc
    import concourse.mybir as mybir
    import concourse.tile as tile
    from concourse.bass_utils import run_bass_kernel_spmd

    f32, bft, fp8, i32 = (
        mybir.dt.float32,
        mybir.dt.bfloat16,
        mybir.dt.float8e4,
        mybir.dt.int32,
    )
    DR = mybir.MatmulPerfMode.DoubleRow

    t_full = np.asarray(inputs["token_embeddings"], dtype=np.float32)
    W = {
        "e": np.asarray(inputs["W_edges"], dtype=np.float32),
        "n": np.asarray(inputs["W_nodes"], dtype=np.float32),
    }
    bias = {
        "e": np.asarray(inputs["b_edges"], dtype=np.float32),
        "n": np.asarray(inputs["b_nodes"], dtype=np.float32),
    }
    gate = {
        "e": np.asarray(inputs["gate_a"], dtype=np.float32).reshape(1, 1),
        "n": np.asarray(inputs["gate_b"], dtype=np.float32).reshape(1, 1),
    }
    t2x = {
        "e": np.asarray(inputs["tokens2edges"], dtype=np.int64),
        "n": np.asarray(inputs["tokens2nodes"], dtype=np.int64),
    }
    x2t = {
        "e": np.asarray(inputs["edges2tokens"], dtype=np.int64),
        "n": np.asarray(inputs["nodes2tokens"], dtype=np.int64),
    }
    ei = {
        "e": np.asarray(inputs["edge_index_edges"], dtype=np.int64),
        "n": np.asarray(inputs["edge_index_nodes"], dtype=np.int64),
    }

    gcns = ("n", "e")  # nodes first: region-B waves stream during agg_n

    # ---- per-sample host index prep (graph renumbering) ----
    samp = []
    for b in range(B):
        sd = {}
        e_img = x2t["e"][b]                      # token of old e-row i
        n_img = x2t["n"][b]
        r_e = np.full(S, -1, np.int64); r_e[e_img] = np.arange(N)
        r_n = np.full(S, -1, np.int64); r_n[n_img] = np.arange(N)
        in_e = r_e >= 0
        in_n = r_n >= 0
        toks_i = np.where(in_e & in_n)[0]        # class i (both images), sorted
        ni = len(toks_i)
        sd["ni"] = ni
        # sigma_e: old e-row -> new position; class i -> 0..ni-1 (token order)
        sig_e = np.full(N, -1, np.int64)
        sig_e[r_e[toks_i]] = np.arange(ni)
        rest_e = np.where(sig_e < 0)[0]          # class ii old rows
        sig_e[rest_e] = ni + np.arange(N - ni)
        # sigma_n: old n-row -> new position; class i -> 0..ni-1 (same order)
        sig_n = np.full(N, -1, np.int64)
        sig_n[r_n[toks_i]] = np.arange(ni)
        rest_n = np.where(sig_n < 0)[0]          # class iii old rows
        sig_n[rest_n] = ni + np.arange(N - ni)
        sd["sig"] = {"e": sig_e, "n": sig_n}
        # token of each NEW position
        inv_e = np.empty(N, np.int64); inv_e[sig_e] = np.arange(N)
        inv_n = np.empty(N, np.int64); inv_n[sig_n] = np.arange(N)
        sd["tokA"] = e_img[inv_e]                # region-A slot j -> token
        sd["tokB"] = n_img[inv_n]                # n-position p -> token
        for g in ("e", "n"):
            eig = sd["sig"][g][ei[g][b]]         # renumbered edge index
            sd[f"A_{g}"], sd[f"deg_{g}"] = _prep_graph(eig)
        sd["seg3_tok"] = np.where(~in_e & ~in_n)[0]
        samp.append(sd)

    LI0 = min(sd["ni"] // P for sd in samp)      # common region-B start chunk
    C = NT + (NT - LI0)                          # output chunks per core

    for b, sd in enumerate(samp):
        ni = sd["ni"]
        # mask[p, c] = 1 iff e_n row (c*128+p) is class i (gets added in region A)
        m = (np.arange(N) < ni).astype(np.float32).reshape(NT, P).T
        sd["mask"] = np.ascontiguousarray(m)
        # t rows in final slot order [4096 region A | (NT-LI0)*128 region B]
        rows = np.zeros((C * P, D), np.float32)
        rows[:N] = t_full[b][sd["tokA"]]
        rows[N:] = t_full[b][sd["tokB"][LI0 * P :]]
        sd["t_rows"] = rows

    # ---- per-core host data ----
    core_data = []
    for b in range(B):
        sd = samp[b]
        for h in range(2):
            d = {}
            for g in gcns:
                xg = t_full[b][t2x[g][b]]        # old x rows
                inv = np.empty(N, np.int64); inv[sd["sig"][g]] = np.arange(N)
                d[f"xT_{g}"] = _xT(xg[inv])      # renumbered row order
                d[f"W_{g}"] = np.ascontiguousarray(
                    W[g][:, h * H : (h + 1) * H].reshape(KT, P, H).transpose(1, 0, 2)
                ).astype(bf16)
                d[f"bias_{g}"] = np.ascontiguousarray(bias[g][None, h * H : (h + 1) * H])
                d[f"gate_{g}"] = gate[g]
                d[f"A_{g}"] = sd[f"A_{g}"]
                d[f"deg_pc_{g}"] = np.ascontiguousarray(
                    sd[f"deg_{g}"].reshape(NT, P).T
                )
            d["mask"] = sd["mask"]
            d["t_seg"] = np.ascontiguousarray(
                sd["t_rows"].reshape(C, P, D).transpose(1, 0, 2)[:, :, h * H : (h + 1) * H]
            ).astype(bf16)
            for g in gcns:
                # piece-contiguous layout: [P, NXC, KT, N/NXC] so each DMA
                # piece is one contiguous run per partition; fp8 where the
                # DoubleRow xw matmul is enabled
                xt = np.ascontiguousarray(
                    d[f"xT_{g}"].reshape(P, KT, NXC, N // NXC).transpose(0, 2, 1, 3)
                )
                if XW_FP8[g]:
                    d[f"xT_{g}"] = xt.astype(f8)
                    d[f"W_{g}"] = d[f"W_{g}"].astype(f8)
                else:
                    d[f"xT_{g}"] = xt
            core_data.append(d)

    # ---- build the SPMD program ----
    nc = bacc.Bacc("TRN2", target_bir_lowering=False, debug=False, num_swdge_queues=1)

    ins_d = {}
    xwdt = {g: (fp8 if XW_FP8[g] else bft) for g in gcns}
    for g in gcns:
        ins_d[f"xT_{g}"] = nc.declare_dram_parameter(
            f"xT_{g}", [P, NXC, KT, N // NXC], xwdt[g], isOutput=False
        )
        ins_d[f"W_{g}"] = nc.declare_dram_parameter(
            f"W_{g}", [P, KT, H], xwdt[g], isOutput=False
        )
        ins_d[f"bias_{g}"] = nc.declare_dram_parameter(f"bias_{g}", [1, H], f32, isOutput=False)
        ins_d[f"gate_{g}"] = nc.declare_dram_parameter(f"gate_{g}", [1, 1], f32, isOutput=False)
        ins_d[f"A_{g}"] = nc.declare_dram_parameter(
            f"A_{g}", [NT, P, NT, P], fp8, isOutput=False
        )
        ins_d[f"deg_pc_{g}"] = nc.declare_dram_parameter(
            f"deg_pc_{g}", [P, NT], i32, isOutput=False
        )
    ins_d["mask"] = nc.declare_dram_parameter("mask", [P, NT], f32, isOutput=False)
    ins_d["t_seg"] = nc.declare_dram_parameter("t_seg", [P, C, H], bft, isOutput=False)
    out_d = nc.declare_dram_parameter("out", [P, C, H], bft, isOutput=True)

    NW = NT // FINW       # 8 e_sb wave-tiles per GCN

    with tile.TileContext(nc) as tc:
        with (
            tc.tile_pool(name="cst", bufs=1) as cst,
            tc.tile_pool(name="xt", bufs=1) as xtp,
            tc.tile_pool(name="yp", bufs=1) as yp,
            tc.tile_pool(name="ap", bufs=4) as apool,
            tc.tile_pool(name="epn", bufs=1) as epn,
            tc.tile_pool(name="epe", bufs=1) as epe,
            tc.tile_pool(name="pwp", bufs=1) as pwp,
            tc.tile_pool(name="tpa", bufs=3) as tpa,
            tc.tile_pool(name="foa", bufs=2) as foa,
            tc.tile_pool(name="psxw", bufs=4, space="PSUM") as psxw,
            tc.tile_pool(name="psz", bufs=3, space="PSUM") as psz,
        ):
            # ---------- setup ----------
            # critical-path first: deg of the first GCN (feeds the dinv chain
            # gating the first xw PSUM drain), then W + xT of the first GCN
            deg_i, Wsb = {}, {}
            for g in gcns:
                deg_i[g] = cst.tile([P, NT], i32, name=f"degi_{g}", tag=f"degi_{g}")
                Wsb[g] = cst.tile([P, KT, H], xwdt[g], name=f"W_{g}", tag=f"W_{g}")
            nc.sync.dma_start(out=Wsb[gcns[0]][:], in_=ins_d[f"W_{gcns[0]}"][:])
            xT_tiles = {
                g: xtp.tile(
                    [P, NXC, KT, N // NXC], xwdt[g], name=f"xT_{g}", tag=f"xT_{g}"
                )
                for g in gcns
            }
            xT0 = xT_tiles[gcns[0]]
            for xc in range(2):
                nc.sync.dma_start(out=xT0[:, xc], in_=ins_d[f"xT_{gcns[0]}"][:, xc])
            nc.sync.dma_start(out=deg_i[gcns[0]][:], in_=ins_d[f"deg_pc_{gcns[0]}"][:])
            # pre-issue the first agg A-blocks of the first GCN: issued here
            # they transfer during the idle early ramp; in the agg loop their
            # issue slot would sit behind ~14us of piece/const issues, leaving
            # the xw->agg boundary at the mercy of DMA jitter
            at_pre = []
            for k in range(2):
                At = apool.tile([P, NT, P], fp8, name="At", tag="At")
                nc.sync.dma_start(out=At[:], in_=ins_d[f"A_{gcns[0]}"][k])
                at_pre.append(At)
            for xc in range(2, NXC):
                nc.sync.dma_start(out=xT0[:, xc], in_=ins_d[f"xT_{gcns[0]}"][:, xc])
            nc.sync.dma_start(out=deg_i[gcns[1]][:], in_=ins_d[f"deg_pc_{gcns[1]}"][:])
            nc.sync.dma_start(out=Wsb[gcns[1]][:], in_=ins_d[f"W_{gcns[1]}"][:])

            gcol, brow = {}, {}
            for g in gcns:
                gcol[g] = cst.tile([P, 1], f32, name=f"gcol_{g}", tag=f"gcol_{g}")
                nc.sync.dma_start(
                    out=gcol[g][:], in_=ins_d[f"gate_{g}"][:1, :].to_broadcast([P, 1])
                )
                brow[g] = cst.tile([P, H], f32, name=f"brow_{g}", tag=f"brow_{g}")
                nc.sync.dma_start(
                    out=brow[g][:], in_=ins_d[f"bias_{g}"][:1, :].to_broadcast([P, H])
                )
            mask = cst.tile([P, NT], f32, name="mask", tag="mask")
            nc.sync.dma_start(out=mask[:], in_=ins_d["mask"][:])

            # dinv = (deg)^-1/2 on vector+scalar; both sqrts adjacent so the
            # SQRT activation table loads once
            dinv, rdeg = {}, {}
            for g in gcns:
                rdeg[g] = cst.tile([P, NT], f32, name=f"rdeg_{g}", tag=f"rdeg_{g}")
                nc.vector.tensor_copy(out=rdeg[g][:], in_=deg_i[g][:])
                nc.vector.reciprocal(rdeg[g][:], rdeg[g][:])
            for g in gcns:
                dinv[g] = cst.tile([P, NT], f32, name=f"dinv_{g}", tag=f"dinv_{g}")
                nc.scalar.sqrt(dinv[g][:], rdeg[g][:])
            bias_ga, dinv_ga = {}, {}
            # gate-dependent constants during the DMA ramp (scalar is idle
            # there); keeps the TANH table load off the drain stream so the
            # COPY table stays loaded through both GCNs' PSUM drains
            for gg in gcns:
                tanh_g = cst.tile([P, 1], f32, name=f"tanh_{gg}", tag=f"tanh_{gg}")
                nc.scalar.activation(
                    out=tanh_g[:], in_=gcol[gg][:],
                    func=mybir.ActivationFunctionType.Tanh,
                )
                bias_ga[gg] = cst.tile([P, H], f32, name=f"biasga_{gg}", tag=f"biasga_{gg}")
                nc.vector.tensor_scalar_mul(bias_ga[gg][:], brow[gg][:], tanh_g[:, :1])
                dinv_ga[gg] = cst.tile([P, NT], f32, name=f"dinvga_{gg}", tag=f"dinvga_{gg}")
                nc.vector.tensor_scalar_mul(dinv_ga[gg][:], dinv[gg][:], tanh_g[:, :1])

            # e_n is one tile (all consumers run after agg_n completes);
            # e_e stays per-wave so region-A waves fire inside the agg_e loop
            en_sb = epn.tile([P, NT, H], bft, name="esb_n", tag="esb_n")
            pw_sb = pwp.tile([P, NT, H], bft, name="pw_sb", tag="pw_sb")
            ee_sb = [
                epe.tile([P, FINW, H], bft, name=f"esb_e_{w}", tag=f"esb_e_{w}")
                for w in range(NW)
            ]

            # ---------- per-GCN ----------
            for gi, g in enumerate(gcns):
                xT = xT_tiles[g]

                y_sb = yp.tile([P, NT, H], fp8, tag="ysb")
                CPX = NT // NXC       # xw chunks per xT piece
                for c in range(NT):
                    pc, lc = c // CPX, c % CPX
                    ps = psxw.tile([P, H], f32)
                    if XW_FP8[g]:
                        for j in range(KT // 2):
                            nc.tensor.matmul(
                                out=ps[:],
                                lhsT=xT[:, pc, 2 * j : 2 * j + 2, lc * P : (lc + 1) * P],
                                rhs=Wsb[g][:, 2 * j : 2 * j + 2, :],
                                start=(j == 0),
                                stop=(j == KT // 2 - 1),
                                perf_mode=DR,
                            )
                    else:
                        for j in range(KT):
                            nc.tensor.matmul(
                                out=ps[:],
                                lhsT=xT[:, pc, j, lc * P : (lc + 1) * P],
                                rhs=Wsb[g][:, j, :],
                                start=(j == 0),
                                stop=(j == KT - 1),
                            )
                    # PSUM drain alternates scalar/vector so neither engine
                    # caps the doubled xw matmul rate; the last chunks split
                    # across both engines so the drain tail (which gates the
                    # first agg tile via the coarse y_sb dependency) collapses
                    if c >= NT - 4:
                        nc.scalar.activation(
                            out=y_sb[:, c, : H // 2],
                            in_=ps[:, : H // 2],
                            func=mybir.ActivationFunctionType.Copy,
                            scale=dinv[g][:, c : c + 1],
                        )
                        nc.vector.tensor_scalar_mul(
                            y_sb[:, c, H // 2 :], ps[:, H // 2 :],
                            dinv[g][:, c : c + 1],
                        )
                    elif c % 2 == 0:
                        nc.scalar.activation(
                            out=y_sb[:, c, :],
                            in_=ps[:],
                            func=mybir.ActivationFunctionType.Copy,
                            scale=dinv[g][:, c : c + 1],
                        )
                    else:
                        nc.vector.tensor_scalar_mul(
                            y_sb[:, c, :], ps[:], dinv[g][:, c : c + 1]
                        )

                # aggregation via dense per-tile adjacency count blocks
                for t_i in range(NT):
                    if gi == 0 and t_i < 2:
                        At = at_pre[t_i]
                    else:
                        At = apool.tile([P, NT, P], fp8, name="At", tag="At")
                        nc.sync.dma_start(out=At[:], in_=ins_d[f"A_{g}"][t_i])
                    if gi == 0 and t_i < NXC:
                        # interleave next GCN's xT piece issues into the agg
                        # loop: the in-order sync sequencer trickles At issues
                        # at agg consumption rate, so issuing all pieces after
                        # the loop would delay xw of the next GCN ~20us
                        g2 = gcns[1]
                        nc.sync.dma_start(
                            out=xT_tiles[g2][:, t_i], in_=ins_d[f"xT_{g2}"][:, t_i]
                        )
                    zt = psz.tile([P, H], f32, name="zt", tag="zt")
                    for j in range(NT // 2):
                        nc.tensor.matmul(
                            out=zt[:],
                            lhsT=At[:, 2 * j : 2 * j + 2, :],
                            rhs=y_sb[:, 2 * j : 2 * j + 2, :],
                            start=(j == 0),
                            stop=(j == NT // 2 - 1),
                            perf_mode=DR,
                        )
                    w, c = t_i // FINW, t_i % FINW
                    e_dst = en_sb[:, t_i, :] if g == "n" else ee_sb[w][:, c, :]
                    nc.vector.scalar_tensor_tensor(
                        out=e_dst,
                        in0=zt[:],
                        scalar=dinv_ga[g][:, t_i : t_i + 1],
                        in1=bias_ga[g][:],
                        op0=mybir.AluOpType.mult,
                        op1=mybir.AluOpType.add,
                    )
                    if g == "n" and c == FINW - 1:
                        # emit region-B wave + region-A pre-sums for this
                        # chunk group now, so the vector queue and the
                        # in-order sync sequencer are both drained by the
                        # end of agg_n (they otherwise delay xw_e/agg_e)
                        if t_i >= LI0:
                            # region-B wave: out = t + e_n for position chunks
                            # [c0, c1) - nodes-only tokens (+ ignored dups)
                            c0, c1 = max(LI0, w * FINW), (w + 1) * FINW
                            o0 = NT + c0 - LI0
                            nw = c1 - c0
                            tch = tpa.tile([P, FINW, H], bft, name="tcha", tag="tcha")
                            nc.sync.dma_start(
                                out=tch[:, :nw, :],
                                in_=ins_d["t_seg"][:, o0 : o0 + nw, :],
                            )
                            och = foa.tile([P, FINW, H], bft, name="ocha", tag="ocha")
                            nc.vector.tensor_tensor(
                                out=och[:, :nw, :],
                                in0=tch[:, :nw, :],
                                in1=en_sb[:, c0:c1, :],
                                op=mybir.AluOpType.add,
                            )
                            nc.gpsimd.dma_start(
                                out=out_d[:, o0 : o0 + nw, :], in_=och[:, :nw, :]
                            )
                        # region-A pre-sum p_w = t + mask*e_n
                        w0 = w * FINW
                        tch = tpa.tile([P, FINW, H], bft, name="tcha", tag="tcha")
                        nc.sync.dma_start(
                            out=tch[:], in_=ins_d["t_seg"][:, w0 : w0 + FINW, :]
                        )
                        for cc in range(FINW):
                            nc.vector.scalar_tensor_tensor(
                                out=pw_sb[:, w0 + cc, :],
                                in0=en_sb[:, w0 + cc, :],
                                scalar=mask[:, w0 + cc : w0 + cc + 1],
                                in1=tch[:, cc, :],
                                op0=mybir.AluOpType.mult,
                                op1=mybir.AluOpType.add,
                            )
                    if g == "e" and w == NW - 1 and c % 2 == 1:
                        # last wave split in two so the final out rows chase
                        # the last agg tiles with minimal tail
                        w0, h0 = w * FINW, c - 1
                        och = foa.tile([P, FINW, H], bft, name="ocha", tag="ocha")
                        nc.vector.tensor_tensor(
                            out=och[:, h0 : h0 + 2, :],
                            in0=pw_sb[:, w0 + h0 : w0 + h0 + 2, :],
                            in1=ee_sb[w][:, h0 : h0 + 2, :],
                            op=mybir.AluOpType.add,
                        )
                        nc.gpsimd.dma_start(
                            out=out_d[:, w0 + h0 : w0 + h0 + 2, :],
                            in_=och[:, h0 : h0 + 2, :],
                        )
                    elif g == "e" and w < NW - 1 and c == FINW - 1:
                        # region-A wave: out = p_w + e_e
                        w0 = w * FINW
                        och = foa.tile([P, FINW, H], bft, name="ocha", tag="ocha")
                        nc.vector.tensor_tensor(
                            out=och[:], in0=pw_sb[:, w0 : w0 + FINW, :],
                            in1=ee_sb[w][:],
                            op=mybir.AluOpType.add,
                        )
                        nc.gpsimd.dma_start(
                            out=out_d[:, w0 : w0 + FINW, :], in_=och[:]
                        )

    nc.compile()

    in_maps = [{k: v for k, v in cd.items()} for cd in core_data]
    global LAST_RESULT
    kw = {}
    if TRACE:
        kw = dict(trace=True, trace_cores=TRACE_CORES, stitch_traces=False)
    res = run_bass_kernel_spmd(nc, in_maps, list(range(8)), **kw)
    LAST_RESULT = res

    out = np.empty((B, S, D), np.float32)
    for b in range(B):
        sd = samp[b]
        ni = sd["ni"]
        tokB_valid = sd["tokB"][ni:]             # class-iii tokens
        boff = N + (ni - LI0 * P)                # their first output row
        for h in range(2):
            o = np.asarray(res.results[2 * b + h]["out"], dtype=np.float32)
            rows = o.transpose(1, 0, 2).reshape(C * P, H)
            out[b, sd["tokA"], h * H : (h + 1) * H] = rows[:N]
            out[b, tokB_valid, h * H : (h + 1) * H] = rows[boff : boff + len(tokB_valid)]
        out[b, sd["seg3_tok"], :] = t_full[b, sd["seg3_tok"], :]
    return out


# revision 54
# speedup vs baseline: 1.0102x; 1.0102x over previous
"""Trainium2 Bass kernel for CausalMessagePassingLayer (2x GCN + gated scatter).

Sharding: 8 cores = 4 samples x 2 halves of the embedding dim (D=768 -> 384).
Each core is fully independent (no collectives).

Pipeline (graph renumbering kills all gathers / DRAM bounces):
  - host renumbers each subgraph's nodes so that tokens present in BOTH
    images sit at the SAME row position in the edges-GCN and nodes-GCN
    output spaces (class-i rows at positions 0..|i|-1 of both).  The final
    combine then reads both GCN outputs as contiguous SBUF tiles - no
    dma_gather, no DRAM roundtrip for e_nodes, no serial valley.
  - xw: y = dinv_src * (x @ W[:, half]); fp8 DoubleRow for the nodes GCN
    (error budget allows one GCN in fp8; max-err elements are e-driven),
    bf16 for the edges GCN; xT is piece-loaded (contiguous-per-partition
    DRAM layout) so the first matmul starts ~12us in; PSUM drains
    alternate scalar/vector; y kept in SBUF as fp8
  - GCN aggregation z[dst] += count * y[src] as dense per-tile adjacency
    blocks (fp8 counts, incl. self-loops) with DoubleRow pairs of src
    tiles accumulating in PSUM; epilogue -> e rows bf16 in SBUF.  This is
    the PE floor; gather-based sparse alternatives lose on DMA descriptor
    rate (~70ns/row/queue).
  - output rows: region A = all 4096 edges-image rows (out = t + e_e +
    mask*e_n, mask zeroes e_n for rows whose token is not in the nodes
    image); region B = nodes-row chunks >= LI0 (out = t + e_n), covering
    all nodes-only tokens (duplicated class-i rows are dropped on host)
  - region-B waves and region-A pre-sums (p = t + mask*e_n) are emitted
    inside the nodes agg loop as each chunk group completes, so the
    vector queue and in-order sync sequencer are drained by agg_n's end;
    each region-A wave inside the edges agg loop is then a single add
    p + e_e; och out-writes are issued from the (otherwise idle) gpsimd
    queue so they never block the sync sequencer's At stream; the next
    GCN's xT piece loads are interleaved into the agg loop for the same
    reason
  - out rows are written bf16 (host converts back to f32); tokens in
    neither image are exact f32 t passthrough assembled on host

Host-side work is restricted to index/descriptor preparation (renumbering,
dense count blocks, degree counts, masks, inverse permutations) and
dtype/layout marshalling of inputs; all floating-point math runs on device.
"""

import numpy as np
import ml_dtypes

B, S, D, N, E = 4, 8192, 768, 4096, 32768
H = D // 2            # per-core half of embedding dim
P = 128
NT = N // P           # 32 node tiles per graph
KT = D // P           # 6 k-tiles of the contraction dim
FINW = 4              # chunks per final wave

NXC = 16              # xT load pieces (256 columns each)
XW_FP8 = {"n": True, "e": False}   # fp8 DoubleRow xw per GCN (error budget)

bf16 = ml_dtypes.bfloat16
f8 = ml_dtypes.float8_e4m3

# test-harness knobs (the grading harness just calls kernel() and these stay default)
TRACE = False
TRACE_CORES = None
LAST_RESULT = None


def _prep_graph(ei):
    """Dense adjacency-count blocks (incl. self loops) + degree counts.

    Returns (A_blocks, deg): A_blocks[t, p, sc, q] = #edges src=sc*128+p ->
    dst=t*128+q, laid out so A_blocks[t] is directly the stack of matmul
    lhsT tiles for dst-tile t. deg includes the self loop.
    """
    s = np.concatenate([ei[0].astype(np.int64), np.arange(N, dtype=np.int64)])
    d = np.concatenate([ei[1].astype(np.int64), np.arange(N, dtype=np.int64)])
    A = np.zeros((N, N), np.float32)
    np.add.at(A, (d, s), 1.0)
    deg = np.bincount(d, minlength=N).astype(np.int32)
    Ab = np.ascontiguousarray(
        A.reshape(NT, P, NT, P).transpose(0, 3, 2, 1)
    ).astype(f8)
    return Ab, deg


def _xT(x):
    """[N, D] -> bf16 [P, KT, N] with K element d = s*128 + p."""
    return np.ascontiguousarray(x.T.reshape(KT, P, N).transpose(1, 0, 2)).astype(bf16)


def kernel(**inputs):
    import concourse.bacc as bacc
    import concourse.mybir as mybir
    import concourse.tile as tile
    from concourse.bass_utils import run_bass_kernel_spmd

    f32, bft, fp8, i32 = (
        mybir.dt.float32,
        mybir.dt.bfloat16,
        mybir.dt.float8e4,
        mybir.dt.int32,
    )
    DR = mybir.MatmulPerfMode.DoubleRow

    t_full = np.asarray(inputs["token_embeddings"], dtype=np.float32)
    W = {
        "e": np.asarray(inputs["W_edges"], dtype=np.float32),
        "n": np.asarray(inputs["W_nodes"], dtype=np.float32),
    }
    bias = {
        "e": np.asarray(inputs["b_edges"], dtype=np.float32),
        "n": np.asarray(inputs["b_nodes"], dtype=np.float32),
    }
    gate = {
        "e": np.asarray(inputs["gate_a"], dtype=np.float32).reshape(1, 1),
        "n": np.asarray(inputs["gate_b"], dtype=np.float32).reshape(1, 1),
    }
    t2x = {
        "e": np.asarray(inputs["tokens2edges"], dtype=np.int64),
        "n": np.asarray(inputs["tokens2nodes"], dtype=np.int64),
    }
    x2t = {
        "e": np.asarray(inputs["edges2tokens"], dtype=np.int64),
        "n": np.asarray(inputs["nodes2tokens"], dtype=np.int64),
    }
    ei = {
        "e": np.asarray(inputs["edge_index_edges"], dtype=np.int64),
        "n": np.asarray(inputs["edge_index_nodes"], dtype=np.int64),
    }

    gcns = ("n", "e")  # nodes first: region-B waves stream during agg_n

    # ---- per-sample host index prep (graph renumbering) ----
    samp = []
    for b in range(B):
        sd = {}
        e_img = x2t["e"][b]                      # token of old e-row i
        n_img = x2t["n"][b]
        r_e = np.full(S, -1, np.int64); r_e[e_img] = np.arange(N)
        r_n = np.full(S, -1, np.int64); r_n[n_img] = np.arange(N)
        in_e = r_e >= 0
        in_n = r_n >= 0
        toks_i = np.where(in_e & in_n)[0]        # class i (both images), sorted
        ni = len(toks_i)
        sd["ni"] = ni
        # sigma_e: old e-row -> new position; class i -> 0..ni-1 (token order)
        sig_e = np.full(N, -1, np.int64)
        sig_e[r_e[toks_i]] = np.arange(ni)
        rest_e = np.where(sig_e < 0)[0]          # class ii old rows
        sig_e[rest_e] = ni + np.arange(N - ni)
        # sigma_n: old n-row -> new position; class i -> 0..ni-1 (same order)
        sig_n = np.full(N, -1, np.int64)
        sig_n[r_n[toks_i]] = np.arange(ni)
        rest_n = np.where(sig_n < 0)[0]          # class iii old rows
        sig_n[rest_n] = ni + np.arange(N - ni)
        sd["sig"] = {"e": sig_e, "n": sig_n}
        # token of each NEW position
        inv_e = np.empty(N, np.int64); inv_e[sig_e] = np.arange(N)
        inv_n = np.empty(N, np.int64); inv_n[sig_n] = np.arange(N)
        sd["tokA"] = e_img[inv_e]                # region-A slot j -> token
        sd["tokB"] = n_img[inv_n]                # n-position p -> token
        for g in ("e", "n"):
            eig = sd["sig"][g][ei[g][b]]         # renumbered edge index
            sd[f"A_{g}"], sd[f"deg_{g}"] = _prep_graph(eig)
        sd["seg3_tok"] = np.where(~in_e & ~in_n)[0]
        samp.append(sd)

    LI0 = min(sd["ni"] // P for sd in samp)      # common region-B start chunk
    C = NT + (NT - LI0)                          # output chunks per core

    for b, sd in enumerate(samp):
        ni = sd["ni"]
        # mask[p, c] = 1 iff e_n row (c*128+p) is class i (gets added in region A)
        m = (np.arange(N) < ni).astype(np.float32).reshape(NT, P).T
        sd["mask"] = np.ascontiguousarray(m)
        # t rows in final slot order [4096 region A | (NT-LI0)*128 region B]
        rows = np.zeros((C * P, D), np.float32)
        rows[:N] = t_full[b][sd["tokA"]]
        rows[N:] = t_full[b][sd["tokB"][LI0 * P :]]
        sd["t_rows"] = rows

    # ---- per-core host data ----
    core_data = []
    for b in range(B):
        sd = samp[b]
        for h in range(2):
            d = {}
            for g in gcns:
                xg = t_full[b][t2x[g][b]]        # old x rows
                inv = np.empty(N, np.int64); inv[sd["sig"][g]] = np.arange(N)
                d[f"xT_{g}"] = _xT(xg[inv])      # renumbered row order
                d[f"W_{g}"] = np.ascontiguousarray(
                    W[g][:, h * H : (h + 1) * H].reshape(KT, P, H).transpose(1, 0, 2)
                ).astype(bf16)
                d[f"bias_{g}"] = np.ascontiguousarray(bias[g][None, h * H : (h + 1) * H])
                d[f"gate_{g}"] = gate[g]
                d[f"A_{g}"] = sd[f"A_{g}"]
                d[f"deg_pc_{g}"] = np.ascontiguousarray(
                    sd[f"deg_{g}"].reshape(NT, P).T
                )
            d["mask"] = sd["mask"]
            d["t_seg"] = np.ascontiguousarray(
                sd["t_rows"].reshape(C, P, D).transpose(1, 0, 2)[:, :, h * H : (h + 1) * H]
            ).astype(bf16)
            for g in gcns:
                # piece-contiguous layout: [P, NXC, KT, N/NXC] so each DMA
                # piece is one contiguous run per partition; fp8 where the
                # DoubleRow xw matmul is enabled
                xt = np.ascontiguousarray(
                    d[f"xT_{g}"].reshape(P, KT, NXC, N // NXC).transpose(0, 2, 1, 3)
                )
                if XW_FP8[g]:
                    d[f"xT_{g}"] = xt.astype(f8)
                    d[f"W_{g}"] = d[f"W_{g}"].astype(f8)
                else:
                    d[f"xT_{g}"] = xt
            core_data.append(d)

    # ---- build the SPMD program ----
    nc = bacc.Bacc("TRN2", target_bir_lowering=False, debug=False, num_swdge_queues=1)

    ins_d = {}
    xwdt = {g: (fp8 if XW_FP8[g] else bft) for g in gcns}
    for g in gcns:
        ins_d[f"xT_{g}"] = nc.declare_dram_parameter(
            f"xT_{g}", [P, NXC, KT, N // NXC], xwdt[g], isOutput=False
        )
        ins_d[f"W_{g}"] = nc.declare_dram_parameter(
            f"W_{g}", [P, KT, H], xwdt[g], isOutput=False
        )
        ins_d[f"bias_{g}"] = nc.declare_dram_parameter(f"bias_{g}", [1, H], f32, isOutput=False)
        ins_d[f"gate_{g}"] = nc.declare_dram_parameter(f"gate_{g}", [1, 1], f32, isOutput=False)
        ins_d[f"A_{g}"] = nc.declare_dram_parameter(
            f"A_{g}", [NT, P, NT, P], fp8, isOutput=False
        )
        ins_d[f"deg_pc_{g}"] = nc.declare_dram_parameter(
            f"deg_pc_{g}", [P, NT], i32, isOutput=False
        )
    ins_d["mask"] = nc.declare_dram_parameter("mask", [P, NT], f32, isOutput=False)
    ins_d["t_seg"] = nc.declare_dram_parameter("t_seg", [P, C, H], bft, isOutput=False)
    out_d = nc.declare_dram_parameter("out", [P, C, H], bft, isOutput=True)

    NW = NT // FINW       # 8 e_sb wave-tiles per GCN

    with tile.TileContext(nc) as tc:
        with (
            tc.tile_pool(name="cst", bufs=1) as cst,
            tc.tile_pool(name="xt", bufs=1) as xtp,
            tc.tile_pool(name="yp", bufs=1) as yp,
            tc.tile_pool(name="ap", bufs=5) as apool,
            tc.tile_pool(name="epn", bufs=1) as epn,
            tc.tile_pool(name="epe", bufs=1) as epe,
            tc.tile_pool(name="pwp", bufs=1) as pwp,
            tc.tile_pool(name="tpa", bufs=3) as tpa,
            tc.tile_pool(name="foa", bufs=2) as foa,
            tc.tile_pool(name="psxw", bufs=4, space="PSUM") as psxw,
            tc.tile_pool(name="psz", bufs=3, space="PSUM") as psz,
        ):
            # ---------- setup ----------
            # critical-path first: deg of the first GCN (feeds the dinv chain
            # gating the first xw PSUM drain), then W + xT of the first GCN
            deg_i, Wsb = {}, {}
            for g in gcns:
                deg_i[g] = cst.tile([P, NT], i32, name=f"degi_{g}", tag=f"degi_{g}")
                Wsb[g] = cst.tile([P, KT, H], xwdt[g], name=f"W_{g}", tag=f"W_{g}")
            nc.sync.dma_start(out=Wsb[gcns[0]][:], in_=ins_d[f"W_{gcns[0]}"][:])
            xT_tiles = {
                g: xtp.tile(
                    [P, NXC, KT, N // NXC], xwdt[g], name=f"xT_{g}", tag=f"xT_{g}"
                )
                for g in gcns
            }
            xT0 = xT_tiles[gcns[0]]
            for xc in range(2):
                nc.sync.dma_start(out=xT0[:, xc], in_=ins_d[f"xT_{gcns[0]}"][:, xc])
            nc.sync.dma_start(out=deg_i[gcns[0]][:], in_=ins_d[f"deg_pc_{gcns[0]}"][:])
            # pre-issue the first agg A-blocks of the first GCN: issued here
            # they transfer during the idle early ramp; in the agg loop their
            # issue slot would sit behind ~14us of piece/const issues, leaving
            # the xw->agg boundary at the mercy of DMA jitter
            at_pre = []
            for k in range(2):
                At = apool.tile([P, NT, P], fp8, name="At", tag="At")
                nc.sync.dma_start(out=At[:], in_=ins_d[f"A_{gcns[0]}"][k])
                at_pre.append(At)
            for xc in range(2, NXC):
                nc.sync.dma_start(out=xT0[:, xc], in_=ins_d[f"xT_{gcns[0]}"][:, xc])
            nc.sync.dma_start(out=deg_i[gcns[1]][:], in_=ins_d[f"deg_pc_{gcns[1]}"][:])
            nc.sync.dma_start(out=Wsb[gcns[1]][:], in_=ins_d[f"W_{gcns[1]}"][:])

            gcol, brow = {}, {}
            for g in gcns:
                gcol[g] = cst.tile([P, 1], f32, name=f"gcol_{g}", tag=f"gcol_{g}")
                nc.sync.dma_start(
                    out=gcol[g][:], in_=ins_d[f"gate_{g}"][:1, :].to_broadcast([P, 1])
                )
                brow[g] = cst.tile([P, H], f32, name=f"brow_{g}", tag=f"brow_{g}")
                nc.sync.dma_start(
                    out=brow[g][:], in_=ins_d[f"bias_{g}"][:1, :].to_broadcast([P, H])
                )
            mask = cst.tile([P, NT], f32, name="mask", tag="mask")
            nc.sync.dma_start(out=mask[:], in_=ins_d["mask"][:])

            # dinv = (deg)^-1/2 on vector+scalar; both sqrts adjacent so the
            # SQRT activation table loads once
            dinv, rdeg = {}, {}
            for g in gcns:
                rdeg[g] = cst.tile([P, NT], f32, name=f"rdeg_{g}", tag=f"rdeg_{g}")
                nc.vector.tensor_copy(out=rdeg[g][:], in_=deg_i[g][:])
                nc.vector.reciprocal(rdeg[g][:], rdeg[g][:])
            for g in gcns:
                dinv[g] = cst.tile([P, NT], f32, name=f"dinv_{g}", tag=f"dinv_{g}")
                nc.scalar.sqrt(dinv[g][:], rdeg[g][:])
            bias_ga, dinv_ga = {}, {}
            # gate-dependent constants during the DMA ramp (scalar is idle
            # there); keeps the TANH table load off the drain stream so the
            # COPY table stays loaded through both GCNs' PSUM drains
            for gg in gcns:
                tanh_g = cst.tile([P, 1], f32, name=f"tanh_{gg}", tag=f"tanh_{gg}")
                nc.scalar.activation(
                    out=tanh_g[:], in_=gcol[gg][:],
                    func=mybir.ActivationFunctionType.Tanh,
                )
                bias_ga[gg] = cst.tile([P, H], f32, name=f"biasga_{gg}", tag=f"biasga_{gg}")
                nc.vector.tensor_scalar_mul(bias_ga[gg][:], brow[gg][:], tanh_g[:, :1])
                dinv_ga[gg] = cst.tile([P, NT], f32, name=f"dinvga_{gg}", tag=f"dinvga_{gg}")
                nc.vector.tensor_scalar_mul(dinv_ga[gg][:], dinv[gg][:], tanh_g[:, :1])

            # e_n is one tile (all consumers run after agg_n completes);
            # e_e stays per-wave so region-A waves fire inside the agg_e loop
            en_sb = epn.tile([P, NT, H], bft, name="esb_n", tag="esb_n")
            pw_sb = pwp.tile([P, NT, H], bft, name="pw_sb", tag="pw_sb")
            ee_sb = [
                epe.tile([P, FINW, H], bft, name=f"esb_e_{w}", tag=f"esb_e_{w}")
                for w in range(NW)
            ]

            # ---------- per-GCN ----------
            for gi, g in enumerate(gcns):
                xT = xT_tiles[g]

                y_sb = yp.tile([P, NT, H], fp8, tag="ysb")
                CPX = NT // NXC       # xw chunks per xT piece
                for c in range(NT):
                    pc, lc = c // CPX, c % CPX
                    ps = psxw.tile([P, H], f32)
                    if XW_FP8[g]:
                        for j in range(KT // 2):
                            nc.tensor.matmul(
                                out=ps[:],
                                lhsT=xT[:, pc, 2 * j : 2 * j + 2, lc * P : (lc + 1) * P],
                                rhs=Wsb[g][:, 2 * j : 2 * j + 2, :],
                                start=(j == 0),
                                stop=(j == KT // 2 - 1),
                                perf_mode=DR,
                            )
                    else:
                        for j in range(KT):
                            nc.tensor.matmul(
                                out=ps[:],
                                lhsT=xT[:, pc, j, lc * P : (lc + 1) * P],
                                rhs=Wsb[g][:, j, :],
                                start=(j == 0),
                                stop=(j == KT - 1),
                            )
                    # PSUM drain alternates scalar/vector so neither engine
                    # caps the doubled xw matmul rate; the last chunks split
                    # across both engines so the drain tail (which gates the
                    # first agg tile via the coarse y_sb dependency) collapses
                    if c >= NT - 4:
                        nc.scalar.activation(
                            out=y_sb[:, c, : H // 2],
                            in_=ps[:, : H // 2],
                            func=mybir.ActivationFunctionType.Copy,
                            scale=dinv[g][:, c : c + 1],
                        )
                        nc.vector.tensor_scalar_mul(
                            y_sb[:, c, H // 2 :], ps[:, H // 2 :],
                            dinv[g][:, c : c + 1],
                        )
                    elif c % 2 == 0:
                        nc.scalar.activation(
                            out=y_sb[:, c, :],
                            in_=ps[:],
                            func=mybir.ActivationFunctionType.Copy,
                            scale=dinv[g][:, c : c + 1],
                        )
                    else:
                        nc.vector.tensor_scalar_mul(
                            y_sb[:, c, :], ps[:], dinv[g][:, c : c + 1]
                        )

                # aggregation via dense per-tile adjacency count blocks
                for t_i in range(NT):
                    if gi == 0 and t_i < 2:
                        At = at_pre[t_i]
                    else:
                        At = apool.tile([P, NT, P], fp8, name="At", tag="At")
                        nc.sync.dma_start(out=At[:], in_=ins_d[f"A_{g}"][t_i])
                    if gi == 0 and t_i < NXC:
                        # interleave next GCN's xT piece issues into the agg
                        # loop: the in-order sync sequencer trickles At issues
                        # at agg consumption rate, so issuing all pieces after
                        # the loop would delay xw of the next GCN ~20us
                        g2 = gcns[1]
                        nc.sync.dma_start(
                            out=xT_tiles[g2][:, t_i], in_=ins_d[f"xT_{g2}"][:, t_i]
                        )
                    zt = psz.tile([P, H], f32, name="zt", tag="zt")
                    for j in range(NT // 2):
                        nc.tensor.matmul(
                            out=zt[:],
                            lhsT=At[:, 2 * j : 2 * j + 2, :],
                            rhs=y_sb[:, 2 * j : 2 * j + 2, :],
                            start=(j == 0),
                            stop=(j == NT // 2 - 1),
                            perf_mode=DR,
                        )
                    w, c = t_i // FINW, t_i % FINW
                    e_dst = en_sb[:, t_i, :] if g == "n" else ee_sb[w][:, c, :]
                    nc.vector.scalar_tensor_tensor(
                        out=e_dst,
                        in0=zt[:],
                        scalar=dinv_ga[g][:, t_i : t_i + 1],
                        in1=bias_ga[g][:],
                        op0=mybir.AluOpType.mult,
                        op1=mybir.AluOpType.add,
                    )
                    if g == "n" and c == FINW - 1:
                        # emit region-B wave + region-A pre-sums for this
                        # chunk group now, so the vector queue and the
                        # in-order sync sequencer are both drained by the
                        # end of agg_n (they otherwise delay xw_e/agg_e)
                        if t_i >= LI0:
                            # region-B wave: out = t + e_n for position chunks
                            # [c0, c1) - nodes-only tokens (+ ignored dups)
                            c0, c1 = max(LI0, w * FINW), (w + 1) * FINW
                            o0 = NT + c0 - LI0
                            nw = c1 - c0
                            tch = tpa.tile([P, FINW, H], bft, name="tcha", tag="tcha")
                            nc.sync.dma_start(
                                out=tch[:, :nw, :],
                                in_=ins_d["t_seg"][:, o0 : o0 + nw, :],
                            )
                            och = foa.tile([P, FINW, H], bft, name="ocha", tag="ocha")
                            nc.vector.tensor_tensor(
                                out=och[:, :nw, :],
                                in0=tch[:, :nw, :],
                                in1=en_sb[:, c0:c1, :],
                                op=mybir.AluOpType.add,
                            )
                            nc.gpsimd.dma_start(
                                out=out_d[:, o0 : o0 + nw, :], in_=och[:, :nw, :]
                            )
                        # region-A pre-sum p_w = t + mask*e_n
                        w0 = w * FINW
                        tch = tpa.tile([P, FINW, H], bft, name="tcha", tag="tcha")
                        nc.sync.dma_start(
                            out=tch[:], in_=ins_d["t_seg"][:, w0 : w0 + FINW, :]
                        )
                        for cc in range(FINW):
                            nc.vector.scalar_tensor_tensor(
                                out=pw_sb[:, w0 + cc, :],
                                in0=en_sb[:, w0 + cc, :],
                                scalar=mask[:, w0 + cc : w0 + cc + 1],
                                in1=tch[:, cc, :],
                                op0=mybir.AluOpType.mult,
                                op1=mybir.AluOpType.add,
                            )
                    if g == "e" and w == NW - 1 and c % 2 == 1:
                        # last wave split in two so the final out rows chase
                        # the last agg tiles with minimal tail
                        w0, h0 = w * FINW, c - 1
                        och = foa.tile([P, FINW, H], bft, name="ocha", tag="ocha")
                        nc.vector.tensor_tensor(
                            out=och[:, h0 : h0 + 2, :],
                            in0=pw_sb[:, w0 + h0 : w0 + h0 + 2, :],
                            in1=ee_sb[w][:, h0 : h0 + 2, :],
                            op=mybir.AluOpType.add,
                        )
                        nc.gpsimd.dma_start(
                            out=out_d[:, w0 + h0 : w0 + h0 + 2, :],
                            in_=och[:, h0 : h0 + 2, :],
                        )
                    elif g == "e" and w < NW - 1 and c == FINW - 1:
                        # region-A wave: out = p_w + e_e
                        w0 = w * FINW
                        och = foa.tile([P, FINW, H], bft, name="ocha", tag="ocha")
                        nc.vector.tensor_tensor(
                            out=och[:], in0=pw_sb[:, w0 : w0 + FINW, :],
                            in1=ee_sb[w][:],
                            op=mybir.AluOpType.add,
                        )
                        nc.gpsimd.dma_start(
                            out=out_d[:, w0 : w0 + FINW, :], in_=och[:]
                        )

    nc.compile()

    in_maps = [{k: v for k, v in cd.items()} for cd in core_data]
    global LAST_RESULT
    kw = {}
    if TRACE:
        kw = dict(trace=True, trace_cores=TRACE_CORES, stitch_traces=False)
    res = run_bass_kernel_spmd(nc, in_maps, list(range(8)), **kw)
    LAST_RESULT = res

    out = np.empty((B, S, D), np.float32)
    for b in range(B):
        sd = samp[b]
        ni = sd["ni"]
        tokB_valid = sd["tokB"][ni:]             # class-iii tokens
        boff = N + (ni - LI0 * P)                # their first output row
        for h in range(2):
            o = np.asarray(res.results[2 * b + h]["out"], dtype=np.float32)
            rows = o.transpose(1, 0, 2).reshape(C * P, H)
            out[b, sd["tokA"], h * H : (h + 1) * H] = rows[:N]
            out[b, tokB_valid, h * H : (h + 1) * H] = rows[boff : boff + len(tokB_valid)]
        out[b, sd["seg3_tok"], :] = t_full[b, sd["seg3_tok"], :]
    return out
